# revision 14
# baseline (speedup 1.0000x reference)
"""Trainium2 Bass kernel for nn_Attention_35021163332119.

Full multi-head attention: qkv = x @ w_qkv; RoPE(q, k); softmax(q k^T / sqrt(dh)) v;
out = heads @ w_out + b_out.  B=2, N=2048, DIM=1024, H=16, DH=64.

Sharding: 8 cores = (batch b in {0,1}) x (head-group g in {0..3} of 4 heads).
Each core computes its 4 heads end-to-end plus the partial output projection
for its head-group's rows of w_out; the host sums the 4 partials per batch
and adds b_out.

On-core layout: x is host-transposed to xT [DIM, N] so the contraction dim
sits on SBUF partitions.  q,k are produced transposed ([dh, n], head pairs
stacked on 128 partitions) straight out of the QKV matmul; v is produced in
natural [n, dh] layout with an extra ones column, so the PV matmul (M=65)
also accumulates the softmax denominator in row 64.  RoPE's interleaved
pair-rotation is a 128x128 +/-1 permutation matmul on the PE plus two DVE
multiplies against cos/sin tables.

Precision: QKV / RoPE / scores matmuls in float32r (full PE rate, ~1e-4
rel err).  The probability-side (exp output, v, output projection) runs in
bf16 — softmax weights and the final linear tolerate it.  Scores matmul
pairs are emitted A,B,A,B across the two stacked heads so the K=64 matmuls
run concurrently in disjoint PE row groups.
"""

import numpy as np

B, N, DIM, H, DH = 2, 2048, 1024, 16, 64
ROPE_BASE = 10000.0
SCALE = DH ** -0.5
N_CORES = 8
G = 4                 # heads per core
KT = DIM // 128       # contraction tiles
NT = N // 128         # sequence tiles
NCH = N // 512        # 512-wide moving chunks

_cache = {}


def _rope_tables():
    inv_freq = (1.0 / (ROPE_BASE ** (np.arange(0, DH, 2, dtype=np.float32) / DH)))
    t = np.arange(N, dtype=np.float32)
    freqs = t[:, None] * inv_freq[None, :]          # [N, DH/2]
    freqs = np.repeat(freqs, 2, axis=-1)            # [N, DH] interleaved
    cosT = np.cos(freqs).T.astype(np.float32)       # [DH, N]
    sinT = np.sin(freqs).T.astype(np.float32)
    cos2 = np.concatenate([cosT, cosT], axis=0)     # [128, N] two heads stacked
    sin2 = np.concatenate([sinT, sinT], axis=0)
    return np.ascontiguousarray(cos2), np.ascontiguousarray(sin2)


def _p2t():
    # rot = P2 @ qT with P2 = blockdiag(P, P), P[2t, 2t+1] = -1, P[2t+1, 2t] = 1
    # matmul computes lhsT.T @ rhs, so pass P2.T
    p = np.zeros((DH, DH), dtype=np.float32)
    for t in range(DH // 2):
        p[2 * t, 2 * t + 1] = -1.0
        p[2 * t + 1, 2 * t] = 1.0
    p2 = np.zeros((128, 128), dtype=np.float32)
    p2[:DH, :DH] = p
    p2[DH:, DH:] = p
    return np.ascontiguousarray(p2.T)


def _build():
    if "nc" in _cache:
        return _cache["nc"]

    import concourse.mybir as mybir
    import concourse.tile as tile
    from concourse import bacc

    F32 = mybir.dt.float32
    F32R = mybir.dt.float32r
    BF16 = mybir.dt.bfloat16
    EXP = mybir.ActivationFunctionType.Exp

    nc = bacc.Bacc("TRN2", target_bir_lowering=False, debug=False)
    xT_d = nc.dram_tensor("xT", [DIM, N], BF16, kind="ExternalInput")
    wqk_d = nc.dram_tensor("wqk", [DIM, 4 * 128], BF16, kind="ExternalInput")
    wv_d = nc.dram_tensor("wv", [DIM, G * DH], BF16, kind="ExternalInput")
    wout_d = nc.dram_tensor("wout", [G * DH, DIM], BF16, kind="ExternalInput")
    cos_d = nc.dram_tensor("cos2", [128, N], F32, kind="ExternalInput")
    sin_d = nc.dram_tensor("sin2", [128, N], F32, kind="ExternalInput")
    p2t_d = nc.dram_tensor("p2t", [128, 128], F32, kind="ExternalInput")
    part_d = nc.dram_tensor("part", [N, DIM], F32, kind="ExternalOutput")

    with tile.TileContext(nc) as tc:
        with tc.tile_pool(name="persist", bufs=1) as persist, \
             tc.tile_pool(name="att", bufs=5) as att, \
             tc.tile_pool(name="norm_w", bufs=2) as norm_w, \
             tc.tile_pool(name="outp", bufs=2) as outp, \
             tc.tile_pool(name="xph", bufs=1) as xph, \
             tc.tile_pool(name="rope_w", bufs=1) as rope_w, \
             tc.tile_pool(name="ps", bufs=3, space="PSUM") as ps, \
             tc.tile_pool(name="pso", bufs=2, space="PSUM") as pso:

            # ---- persistent tiles ----
            qk_sb = [persist.tile([128, N], F32R, tag=f"qk{m}", name=f"qk{m}")
                     for m in range(4)]          # q01T, q23T, k01T, k23T
            v_aug = persist.tile([128, NT, G, DH + 1], BF16, tag="vaug")
            wout_sb = [persist.tile([128, DIM], BF16, tag=f"wo{kk}", name=f"wo{kk}")
                       for kk in range(2)]
            outT = [persist.tile([128, N], BF16, tag=f"outT{p}", name=f"outT{p}")
                    for p in range(2)]

            # ---- phase-1 tiles ----
            xT = [xph.tile([128, N], BF16, tag=f"xT{k}", name=f"xT{k}")
                  for k in range(KT)]
            wqk = [xph.tile([128, 4 * 128], BF16, tag=f"wqk{k}", name=f"wqk{k}")
                   for k in range(KT)]
            wv = [xph.tile([128, G * DH], BF16, tag=f"wv{k}", name=f"wv{k}")
                  for k in range(KT)]
            cos2 = xph.tile([128, N], F32R, tag="cos2")
            sin2 = xph.tile([128, N], F32R, tag="sin2")
            p2t = xph.tile([128, 128], F32R, tag="p2t")
            ones_col = xph.tile([128, NT, G, 1], F32, tag="ones")

            for kk in range(2):
                nc.sync.dma_start(
                    out=wout_sb[kk],
                    in_=wout_d.ap().rearrange("(t p) m -> t p m", p=128)[kk])
            nc.sync.dma_start(out=cos2, in_=cos_d.ap().bitcast(F32R))
            nc.sync.dma_start(out=sin2, in_=sin_d.ap().bitcast(F32R))
            nc.sync.dma_start(out=p2t, in_=p2t_d.ap().bitcast(F32R))
            nc.vector.memset(ones_col, 1.0)
            for k in range(KT):
                for q in range(2):
                    qsl = slice(q * (N // 2), (q + 1) * (N // 2))
                    nc.sync.dma_start(
                        out=xT[k][:, qsl],
                        in_=xT_d.ap().rearrange(
                            "(t p) n -> t p n", p=128)[k][:, qsl])
                nc.sync.dma_start(
                    out=wqk[k],
                    in_=wqk_d.ap().rearrange("(t p) m -> t p m", p=128)[k])
                nc.sync.dma_start(
                    out=wv[k],
                    in_=wv_d.ap().rearrange("(t p) m -> t p m", p=128)[k])

            def qk_all():
                # all four q/k pair tiles; PSUM accumulators processed in
                # groups of 3 with the contraction loop OUTER, so the first
                # group's matmuls pace with the xT DMA arrivals instead of
                # waiting for the full tensor
                jobs = [(m, c2) for m in range(4) for c2 in range(2)]
                for g0 in range(0, len(jobs), 3):
                    group = jobs[g0:g0 + 3]
                    tiles = {}
                    for m, c2 in group:
                        tiles[(m, c2)] = ps.tile([128, 1024], F32, tag="s",
                                                 name=f"qk{m}{c2}")
                    for k in range(KT):
                        for m, c2 in group:
                            for half in range(2):
                                hsl = slice(half * 512, (half + 1) * 512)
                                csl = slice(c2 * 1024 + half * 512,
                                            c2 * 1024 + (half + 1) * 512)
                                nc.tensor.matmul(
                                    tiles[(m, c2)][:, hsl],
                                    wqk[k][:, m * 128:(m + 1) * 128],
                                    xT[k][:, csl],
                                    start=(k == 0), stop=(k == KT - 1))
                    for m, c2 in group:
                        nc.scalar.copy(
                            qk_sb[m][:, c2 * 1024:(c2 + 1) * 1024],
                            tiles[(m, c2)])

            def rope_pair(p):
                for m in (p, 2 + p):
                    tmp = rope_w.tile([128, N], F32R, tag="ropetmp")
                    for c2 in range(2):
                        rot_ps = ps.tile([128, 1024], F32, tag="s",
                                         name="mm_rot")
                        for half in range(2):
                            csl = slice(c2 * 1024 + half * 512,
                                        c2 * 1024 + (half + 1) * 512)
                            nc.tensor.matmul(
                                rot_ps[:, half * 512:(half + 1) * 512],
                                p2t, qk_sb[m][:, csl],
                                start=True, stop=True)
                        nc.vector.tensor_mul(
                            tmp[:, c2 * 1024:(c2 + 1) * 1024], rot_ps,
                            sin2[:, c2 * 1024:(c2 + 1) * 1024])
                    nc.vector.tensor_mul(qk_sb[m], qk_sb[m], cos2)
                    nc.vector.tensor_add(qk_sb[m], qk_sb[m], tmp)

            def v_all():
                for tn in range(NT):
                    mm_ps = ps.tile([128, 1024], F32, tag="s", name="mm_v")
                    for k in range(KT):
                        nc.tensor.matmul(
                            mm_ps[:, 0:G * DH],
                            xT[k][:, tn * 128:(tn + 1) * 128],
                            wv[k],
                            start=(k == 0), stop=(k == KT - 1))
                    nc.vector.tensor_copy(
                        v_aug[:, tn, :, 0:DH],
                        mm_ps[:, 0:G * DH].rearrange("p (h d) -> p h d", h=G))
                nc.vector.tensor_copy(v_aug[:, :, :, DH:DH + 1], ones_col)

            def attention(p, iq):
                """One (head-pair, i-quarter of 512) block.  Scores PSUM
                tiles hold two j-tiles x 512 i-columns per head, so each exp
                op still covers 1024 elements while the PV accumulators only
                need one PSUM bank per head (leaving 3 scores buffers for
                pipeline slack)."""
                qT = qk_sb[p]
                kTt = qk_sb[2 + p]
                i0 = iq * 512
                isl = slice(i0, i0 + 512)
                o_ps = [pso.tile([DH + 1, 512], F32, tag="o", name=f"o{hh}")
                        for hh in range(2)]

                def emit_pv(jj, exps):
                    for hh in range(2):
                        for half in range(2):
                            j = 2 * jj + half
                            nc.tensor.matmul(
                                o_ps[hh],
                                v_aug[:, j, 2 * p + hh, :],
                                exps[hh][:, half * 512:(half + 1) * 512],
                                start=(j == 0), stop=(j == NT - 1))

                pend = None   # software pipeline: PV of jj-1 runs while exp
                for jj in range(NT // 2):   # of jj occupies the scalar engine
                    s_ps = [ps.tile([128, 1024], F32, tag="s", name=f"s{hh}")
                            for hh in range(2)]
                    # scores: interleave heads A,B,A,B -> disjoint PE row
                    # groups run concurrently
                    for half in range(2):
                        j = 2 * jj + half
                        jsl = slice(j * 128, (j + 1) * 128)
                        for hh in range(2):
                            hsl = slice(hh * DH, (hh + 1) * DH)
                            nc.tensor.matmul(
                                s_ps[hh][:, half * 512:(half + 1) * 512],
                                kTt[hsl, jsl], qT[hsl, isl],
                                start=True, stop=True)
                    exps = []
                    for hh in range(2):
                        expT = att.tile([128, 1024], BF16, tag="exp")
                        nc.scalar.activation(expT, s_ps[hh], EXP, scale=SCALE)
                        exps.append(expT)
                    if pend is not None:
                        emit_pv(jj - 1, pend)
                    pend = exps
                emit_pv(NT // 2 - 1, pend)
                # move PV accumulators to SBUF so PSUM frees immediately;
                # normalization happens off the critical path
                for hh in range(2):
                    o_sb = norm_w.tile([DH + 1, 512], F32, tag=f"osb{hh}",
                                       name=f"osb{hh}")
                    nc.vector.tensor_copy(o_sb, o_ps[hh])
                    recip0 = norm_w.tile([1, 512], F32, tag=f"r0{hh}",
                                         name=f"r0{hh}")
                    nc.sync.dma_start(out=recip0, in_=o_sb[DH:DH + 1, :])
                    nc.vector.reciprocal_approx_fast(recip0, recip0)
                    bc = norm_w.tile([DH, 512], F32, tag=f"bc{hh}",
                                     name=f"bc{hh}")
                    nc.gpsimd.partition_broadcast(bc, recip0)
                    if hh == 0:
                        nc.vector.tensor_mul(outT[p][0:DH, isl],
                                             o_sb[0:DH, :], bc)
                    else:
                        tmpb = norm_w.tile([DH, 512], BF16, tag="tmpb")
                        nc.vector.tensor_mul(tmpb, o_sb[0:DH, :], bc)
                        nc.sync.dma_start(out=outT[p][DH:2 * DH, isl],
                                          in_=tmpb)

            def proj_tile(tn):
                nsl = slice(tn * 128, (tn + 1) * 128)
                f_ps = ps.tile([128, 1024], F32, tag="s", name="f_ps")
                for c2 in range(2):
                    c2sl = slice(c2 * 512, (c2 + 1) * 512)
                    for kk in range(2):
                        nc.tensor.matmul(
                            f_ps[:, c2sl],
                            outT[kk][:, nsl], wout_sb[kk][:, c2sl],
                            start=(kk == 0), stop=(kk == 1))
                out_sb = outp.tile([128, DIM], F32, tag="osb")
                if tn % 2 == 0:
                    nc.scalar.copy(out_sb, f_ps)
                else:
                    nc.vector.tensor_copy(out_sb, f_ps)
                nc.sync.dma_start(
                    out=part_d.ap().rearrange("(t p) m -> t p m", p=128)[tn],
                    in_=out_sb)

            # ---- emission order ----
            qk_all()
            rope_pair(0)
            rope_pair(1)
            v_all()
            for p in range(2):
                for iq in range(4):
                    attention(p, iq)
            for tn in range(NT):
                proj_tile(tn)
    nc.compile()
    _cache["nc"] = nc
    return nc


def kernel(x, w_qkv, w_out, b_out, _trace=False):
    import ml_dtypes
    from concourse.bass_utils import run_bass_kernel_spmd

    x = np.asarray(x, dtype=np.float32)
    w_qkv = np.asarray(w_qkv, dtype=np.float32)
    w_out = np.asarray(w_out, dtype=np.float32)
    b_out = np.asarray(b_out, dtype=np.float32)

    cos2, sin2 = _rope_tables()
    p2t = _p2t()

    in_maps = []
    for c in range(N_CORES):
        b, g = divmod(c, G)
        cols = []
        for blk in range(2):                      # q block, k block
            base = blk * H * DH + g * G * DH
            cols.append(w_qkv[:, base:base + G * DH])
        wqk_c = np.ascontiguousarray(np.concatenate(cols, axis=1))  # [DIM, 512]
        wv_c = np.ascontiguousarray(
            w_qkv[:, 2 * H * DH + g * G * DH: 2 * H * DH + (g + 1) * G * DH])
        wout_c = np.ascontiguousarray(
            w_out[g * G * DH:(g + 1) * G * DH, :]).astype(ml_dtypes.bfloat16)
        in_maps.append({
            "xT": np.ascontiguousarray(x[b].T).astype(ml_dtypes.bfloat16),
            "wqk": wqk_c.astype(ml_dtypes.bfloat16),
            "wv": wv_c.astype(ml_dtypes.bfloat16),
            "wout": wout_c,
            "cos2": cos2,
            "sin2": sin2,
            "p2t": p2t,
        })

    nc = _build()
    res = run_bass_kernel_spmd(nc, in_maps, core_ids=list(range(N_CORES)),
                               trace=_trace)
    out = np.empty((B, N, DIM), dtype=np.float32)
    for b in range(B):
        acc = res.results[G * b]["part"].copy()
        for g in range(1, G):
            acc += res.results[G * b + g]["part"]
        out[b] = acc + b_out
    if _trace:
        kernel.last_results = res
    return out


# revision 15
# speedup vs baseline: 1.0364x; 1.0364x over previous
"""Trainium2 Bass kernel for nn_Attention_35021163332119.

Full multi-head attention: qkv = x @ w_qkv; RoPE(q, k); softmax(q k^T / sqrt(dh)) v;
out = heads @ w_out + b_out.  B=2, N=2048, DIM=1024, H=16, DH=64.

Sharding: 8 cores = (batch b in {0,1}) x (head-group g in {0..3} of 4 heads).
Each core computes its 4 heads end-to-end plus the partial output projection
for its head-group's rows of w_out; the host sums the 4 partials per batch
and adds b_out.

On-core layout: x is host-transposed to xT [DIM, N] so the contraction dim
sits on SBUF partitions.  q,k are produced transposed ([dh, n], head pairs
stacked on 128 partitions) straight out of the QKV matmul; v is produced in
natural [n, dh] layout with an extra ones column, so the PV matmul (M=65)
also accumulates the softmax denominator in row 64.  RoPE's interleaved
pair-rotation is a 128x128 +/-1 permutation matmul on the PE plus two DVE
multiplies against cos/sin tables.

Precision: QKV / RoPE / scores matmuls in float32r (full PE rate, ~1e-4
rel err).  The probability-side (exp output, v, output projection) runs in
bf16 — softmax weights and the final linear tolerate it.  Scores matmul
pairs are emitted A,B,A,B across the two stacked heads so the K=64 matmuls
run concurrently in disjoint PE row groups.
"""

import numpy as np

B, N, DIM, H, DH = 2, 2048, 1024, 16, 64
ROPE_BASE = 10000.0
SCALE = DH ** -0.5
N_CORES = 8
G = 4                 # heads per core
KT = DIM // 128       # contraction tiles
NT = N // 128         # sequence tiles
NCH = N // 512        # 512-wide moving chunks

_cache = {}


def _rope_tables():
    inv_freq = (1.0 / (ROPE_BASE ** (np.arange(0, DH, 2, dtype=np.float32) / DH)))
    t = np.arange(N, dtype=np.float32)
    freqs = t[:, None] * inv_freq[None, :]          # [N, DH/2]
    freqs = np.repeat(freqs, 2, axis=-1)            # [N, DH] interleaved
    cosT = np.cos(freqs).T.astype(np.float32)       # [DH, N]
    sinT = np.sin(freqs).T.astype(np.float32)
    cos2 = np.concatenate([cosT, cosT], axis=0)     # [128, N] two heads stacked
    sin2 = np.concatenate([sinT, sinT], axis=0)
    return np.ascontiguousarray(cos2), np.ascontiguousarray(sin2)


def _p2t():
    # rot = P2 @ qT with P2 = blockdiag(P, P), P[2t, 2t+1] = -1, P[2t+1, 2t] = 1
    # matmul computes lhsT.T @ rhs, so pass P2.T
    p = np.zeros((DH, DH), dtype=np.float32)
    for t in range(DH // 2):
        p[2 * t, 2 * t + 1] = -1.0
        p[2 * t + 1, 2 * t] = 1.0
    p2 = np.zeros((128, 128), dtype=np.float32)
    p2[:DH, :DH] = p
    p2[DH:, DH:] = p
    return np.ascontiguousarray(p2.T)


def _build():
    if "nc" in _cache:
        return _cache["nc"]

    import concourse.mybir as mybir
    import concourse.tile as tile
    from concourse import bacc

    F32 = mybir.dt.float32
    F32R = mybir.dt.float32r
    BF16 = mybir.dt.bfloat16
    EXP = mybir.ActivationFunctionType.Exp

    nc = bacc.Bacc("TRN2", target_bir_lowering=False, debug=False)
    xT_d = nc.dram_tensor("xT", [DIM, N], BF16, kind="ExternalInput")
    wqk_d = nc.dram_tensor("wqk", [DIM, 4 * 128], BF16, kind="ExternalInput")
    wv_d = nc.dram_tensor("wv", [DIM, G * DH], BF16, kind="ExternalInput")
    wout_d = nc.dram_tensor("wout", [G * DH, DIM], BF16, kind="ExternalInput")
    cos_d = nc.dram_tensor("cos2", [128, N], F32, kind="ExternalInput")
    sin_d = nc.dram_tensor("sin2", [128, N], F32, kind="ExternalInput")
    p2t_d = nc.dram_tensor("p2t", [128, 128], F32, kind="ExternalInput")
    part_d = nc.dram_tensor("part", [N, DIM], F32, kind="ExternalOutput")

    with tile.TileContext(nc) as tc:
        with tc.tile_pool(name="persist", bufs=1) as persist, \
             tc.tile_pool(name="att", bufs=5) as att, \
             tc.tile_pool(name="norm_w", bufs=2) as norm_w, \
             tc.tile_pool(name="outp", bufs=2) as outp, \
             tc.tile_pool(name="xph", bufs=1) as xph, \
             tc.tile_pool(name="rope_w", bufs=1) as rope_w, \
             tc.tile_pool(name="ps", bufs=3, space="PSUM") as ps, \
             tc.tile_pool(name="pso", bufs=2, space="PSUM") as pso:

            # ---- persistent tiles ----
            qk_sb = [persist.tile([128, N], F32R, tag=f"qk{m}", name=f"qk{m}")
                     for m in range(4)]          # q01T, q23T, k01T, k23T
            v_aug = persist.tile([128, NT, G, DH + 1], BF16, tag="vaug")
            wout_sb = [persist.tile([128, DIM], BF16, tag=f"wo{kk}", name=f"wo{kk}")
                       for kk in range(2)]
            outT = [persist.tile([128, N], BF16, tag=f"outT{p}", name=f"outT{p}")
                    for p in range(2)]

            # ---- phase-1 tiles ----
            xT = [xph.tile([128, N], BF16, tag=f"xT{k}", name=f"xT{k}")
                  for k in range(KT)]
            wqk = [xph.tile([128, 4 * 128], BF16, tag=f"wqk{k}", name=f"wqk{k}")
                   for k in range(KT)]
            wv = [xph.tile([128, G * DH], BF16, tag=f"wv{k}", name=f"wv{k}")
                  for k in range(KT)]
            cos2 = xph.tile([128, N], F32R, tag="cos2")
            sin2 = xph.tile([128, N], F32R, tag="sin2")
            p2t = xph.tile([128, 128], F32R, tag="p2t")
            ones_col = xph.tile([128, NT, G, 1], F32, tag="ones")

            for kk in range(2):
                nc.sync.dma_start(
                    out=wout_sb[kk],
                    in_=wout_d.ap().rearrange("(t p) m -> t p m", p=128)[kk])
            nc.sync.dma_start(out=cos2, in_=cos_d.ap().bitcast(F32R))
            nc.sync.dma_start(out=sin2, in_=sin_d.ap().bitcast(F32R))
            nc.sync.dma_start(out=p2t, in_=p2t_d.ap().bitcast(F32R))
            nc.vector.memset(ones_col, 1.0)
            for k in range(KT):
                for q in range(2):
                    qsl = slice(q * (N // 2), (q + 1) * (N // 2))
                    nc.sync.dma_start(
                        out=xT[k][:, qsl],
                        in_=xT_d.ap().rearrange(
                            "(t p) n -> t p n", p=128)[k][:, qsl])
                nc.sync.dma_start(
                    out=wqk[k],
                    in_=wqk_d.ap().rearrange("(t p) m -> t p m", p=128)[k])
                nc.sync.dma_start(
                    out=wv[k],
                    in_=wv_d.ap().rearrange("(t p) m -> t p m", p=128)[k])

            def qk_pair(p):
                # q pair tile (m=p) and k pair tile (m=2+p); two 512-chunks
                # per PSUM tile so the PSUM->SBUF copies are [128, 1024]
                for m in (p, 2 + p):
                    for c2 in range(2):
                        mm_ps = ps.tile([128, 1024], F32, tag="s", name="mm_qk")
                        for half in range(2):
                            hsl = slice(half * 512, (half + 1) * 512)
                            csl = slice(c2 * 1024 + half * 512,
                                        c2 * 1024 + (half + 1) * 512)
                            for k in range(KT):
                                nc.tensor.matmul(
                                    mm_ps[:, hsl],
                                    wqk[k][:, m * 128:(m + 1) * 128],
                                    xT[k][:, csl],
                                    start=(k == 0), stop=(k == KT - 1))
                        nc.scalar.copy(
                            qk_sb[m][:, c2 * 1024:(c2 + 1) * 1024], mm_ps)

            def rope_pair(p):
                for m in (p, 2 + p):
                    tmp = rope_w.tile([128, N], F32R, tag="ropetmp")
                    for c2 in range(2):
                        rot_ps = ps.tile([128, 1024], F32, tag="s",
                                         name="mm_rot")
                        for half in range(2):
                            csl = slice(c2 * 1024 + half * 512,
                                        c2 * 1024 + (half + 1) * 512)
                            nc.tensor.matmul(
                                rot_ps[:, half * 512:(half + 1) * 512],
                                p2t, qk_sb[m][:, csl],
                                start=True, stop=True)
                        nc.vector.tensor_mul(
                            tmp[:, c2 * 1024:(c2 + 1) * 1024], rot_ps,
                            sin2[:, c2 * 1024:(c2 + 1) * 1024])
                    nc.vector.tensor_mul(qk_sb[m], qk_sb[m], cos2)
                    nc.vector.tensor_add(qk_sb[m], qk_sb[m], tmp)

            def v_all():
                for tn in range(NT):
                    mm_ps = ps.tile([128, 1024], F32, tag="s", name="mm_v")
                    for k in range(KT):
                        nc.tensor.matmul(
                            mm_ps[:, 0:G * DH],
                            xT[k][:, tn * 128:(tn + 1) * 128],
                            wv[k],
                            start=(k == 0), stop=(k == KT - 1))
                    nc.vector.tensor_copy(
                        v_aug[:, tn, :, 0:DH],
                        mm_ps[:, 0:G * DH].rearrange("p (h d) -> p h d", h=G))
                nc.vector.tensor_copy(v_aug[:, :, :, DH:DH + 1], ones_col)

            def attention(p, iq):
                """One (head-pair, i-quarter of 512) block.  Scores PSUM
                tiles hold two j-tiles x 512 i-columns per head, so each exp
                op still covers 1024 elements while the PV accumulators only
                need one PSUM bank per head (leaving 3 scores buffers for
                pipeline slack)."""
                qT = qk_sb[p]
                kTt = qk_sb[2 + p]
                i0 = iq * 512
                isl = slice(i0, i0 + 512)
                o_ps = [pso.tile([DH + 1, 512], F32, tag="o", name=f"o{hh}")
                        for hh in range(2)]

                def emit_pv(jj, exps):
                    for hh in range(2):
                        for half in range(2):
                            j = 2 * jj + half
                            nc.tensor.matmul(
                                o_ps[hh],
                                v_aug[:, j, 2 * p + hh, :],
                                exps[hh][:, half * 512:(half + 1) * 512],
                                start=(j == 0), stop=(j == NT - 1))

                pend = None   # software pipeline: PV of jj-1 runs while exp
                for jj in range(NT // 2):   # of jj occupies the scalar engine
                    s_ps = [ps.tile([128, 1024], F32, tag="s", name=f"s{hh}")
                            for hh in range(2)]
                    # scores: interleave heads A,B,A,B -> disjoint PE row
                    # groups run concurrently
                    for half in range(2):
                        j = 2 * jj + half
                        jsl = slice(j * 128, (j + 1) * 128)
                        for hh in range(2):
                            hsl = slice(hh * DH, (hh + 1) * DH)
                            nc.tensor.matmul(
                                s_ps[hh][:, half * 512:(half + 1) * 512],
                                kTt[hsl, jsl], qT[hsl, isl],
                                start=True, stop=True)
                    exps = []
                    for hh in range(2):
                        expT = att.tile([128, 1024], BF16, tag="exp")
                        nc.scalar.activation(expT, s_ps[hh], EXP, scale=SCALE)
                        exps.append(expT)
                    if pend is not None:
                        emit_pv(jj - 1, pend)
                    pend = exps
                emit_pv(NT // 2 - 1, pend)
                # move PV accumulators to SBUF so PSUM frees immediately;
                # normalization happens off the critical path
                for hh in range(2):
                    o_sb = norm_w.tile([DH + 1, 512], F32, tag=f"osb{hh}",
                                       name=f"osb{hh}")
                    nc.vector.tensor_copy(o_sb, o_ps[hh])
                    recip0 = norm_w.tile([1, 512], F32, tag=f"r0{hh}",
                                         name=f"r0{hh}")
                    nc.sync.dma_start(out=recip0, in_=o_sb[DH:DH + 1, :])
                    nc.vector.reciprocal_approx_fast(recip0, recip0)
                    bc = norm_w.tile([DH, 512], F32, tag=f"bc{hh}",
                                     name=f"bc{hh}")
                    nc.gpsimd.partition_broadcast(bc, recip0)
                    if hh == 0:
                        nc.vector.tensor_mul(outT[p][0:DH, isl],
                                             o_sb[0:DH, :], bc)
                    else:
                        tmpb = norm_w.tile([DH, 512], BF16, tag="tmpb")
                        nc.vector.tensor_mul(tmpb, o_sb[0:DH, :], bc)
                        nc.sync.dma_start(out=outT[p][DH:2 * DH, isl],
                                          in_=tmpb)

            def proj_tile(tn):
                nsl = slice(tn * 128, (tn + 1) * 128)
                f_ps = ps.tile([128, 1024], F32, tag="s", name="f_ps")
                for c2 in range(2):
                    c2sl = slice(c2 * 512, (c2 + 1) * 512)
                    for kk in range(2):
                        nc.tensor.matmul(
                            f_ps[:, c2sl],
                            outT[kk][:, nsl], wout_sb[kk][:, c2sl],
                            start=(kk == 0), stop=(kk == 1))
                out_sb = outp.tile([128, DIM], F32, tag="osb")
                if tn % 2 == 0:
                    nc.scalar.copy(out_sb, f_ps)
                else:
                    nc.vector.tensor_copy(out_sb, f_ps)
                nc.sync.dma_start(
                    out=part_d.ap().rearrange("(t p) m -> t p m", p=128)[tn],
                    in_=out_sb)

            # ---- emission order ----
            qk_pair(0)
            rope_pair(0)
            qk_pair(1)
            rope_pair(1)
            v_all()
            for p in range(2):
                for iq in range(4):
                    attention(p, iq)
            for tn in range(NT):
                proj_tile(tn)
    nc.compile()
    _cache["nc"] = nc
    return nc


def kernel(x, w_qkv, w_out, b_out, _trace=False):
    import ml_dtypes
    from concourse.bass_utils import run_bass_kernel_spmd

    x = np.asarray(x, dtype=np.float32)
    w_qkv = np.asarray(w_qkv, dtype=np.float32)
    w_out = np.asarray(w_out, dtype=np.float32)
    b_out = np.asarray(b_out, dtype=np.float32)

    cos2, sin2 = _rope_tables()
    p2t = _p2t()

    in_maps = []
    for c in range(N_CORES):
        b, g = divmod(c, G)
        cols = []
        for blk in range(2):                      # q block, k block
            base = blk * H * DH + g * G * DH
            cols.append(w_qkv[:, base:base + G * DH])
        wqk_c = np.ascontiguousarray(np.concatenate(cols, axis=1))  # [DIM, 512]
        wv_c = np.ascontiguousarray(
            w_qkv[:, 2 * H * DH + g * G * DH: 2 * H * DH + (g + 1) * G * DH])
        wout_c = np.ascontiguousarray(
            w_out[g * G * DH:(g + 1) * G * DH, :]).astype(ml_dtypes.bfloat16)
        in_maps.append({
            "xT": np.ascontiguousarray(x[b].T).astype(ml_dtypes.bfloat16),
            "wqk": wqk_c.astype(ml_dtypes.bfloat16),
            "wv": wv_c.astype(ml_dtypes.bfloat16),
            "wout": wout_c,
            "cos2": cos2,
            "sin2": sin2,
            "p2t": p2t,
        })

    nc = _build()
    res = run_bass_kernel_spmd(nc, in_maps, core_ids=list(range(N_CORES)),
                               trace=_trace)
    out = np.empty((B, N, DIM), dtype=np.float32)
    for b in range(B):
        acc = res.results[G * b]["part"].copy()
        for g in range(1, G):
            acc += res.results[G * b + g]["part"]
        out[b] = acc + b_out
    if _trace:
        kernel.last_results = res
    return out
